# revision 1
# baseline (speedup 1.0000x reference)
"""DVH global loss (histogram binning) Trainium2 kernel.

Strategy: 8 cores, data-parallel over (batch, voxel-half): core = 2*b + h.
Each core computes a joint 16x32 (q, r) histogram of the dose-bin index
j = floor-ish(d * 499/75) (j = searchsorted(linspace(0,75,500), d*m,
'right') - 1 up to ulp-boundary noise), via exact fp32 magic-number
rounding chains split across DVE and ACT; bf16 one-hot expansion on DVE;
PE accumulates outer(A_col, B_col) over all voxel columns into PSUM[16,32].
Masked voxels are pushed past bin 4000 so their q >= 125 misses the 16-wide
q-one-hot entirely (counts only unmasked voxels). Host combines: signed
hist e = H_pred - H_gt per batch, reverse-cumsum -> DVH count differences,
MSE over (batch, bin) with per-batch denom = sum(mask) + 1e-6. Histogram
counts are integer-exact in fp32.

A post-Tile pass legalizes semaphore waits: trn2 engine instructions have
very few sync-wait slots (TensorTensor/DMA structs fit one), so redundant
same-engine waits are dropped (engine queues are strict in-order FIFO) and
excess waits move onto same-engine NOPs inserted before the instruction.
"""

import sys
from contextlib import ExitStack

if "/opt/trn_rl_repo" not in sys.path:
    sys.path.insert(0, "/opt/trn_rl_repo")

import numpy as np

import concourse.bass as bass
import concourse.tile as tile
from concourse import mybir
from concourse.bass_utils import run_bass_kernel_spmd

F32 = mybir.dt.float32
BF16 = mybir.dt.bfloat16

C1 = 499.0 / 75.0
GUARD = 0.4998
U2_S1 = -4000.0 / C1  # * m
U2_S2 = (4000.0 - GUARD) / C1  # + const


# trn2 engine instructions have very few sync-wait slots (TT has one). Tile
# emits redundant same-engine waits and multi-waits that walrus rejects.
# Legalize: drop own-engine-sem waits on in-order compute engines, then move
# excess waits onto earlier same-engine instructions with free slots.
_ENGINE_SEM_PREFIX = {
    mybir.EngineType.DVE: "DVE_",
    mybir.EngineType.Activation: "Activation_",
    mybir.EngineType.Pool: "Pool_",
}
_COMPUTE_ENGINES = (
    mybir.EngineType.DVE,
    mybir.EngineType.Activation,
    mybir.EngineType.Pool,
    mybir.EngineType.PE,
)


_EXEMPT_TYPES = (
    "InstCall",
    "InstUnconditionalBranch",
    "InstRegisterMove",
    "InstISA",
    "InstNoOp",
)

_SELF_DROP_TYPES = (
    "InstTensorTensor",
    "InstTensorScalarPtr",
    "InstTensorReduce",
    "InstActivation",
    "InstMemset",
    "InstTensorCopy",
)


def legalize_sync_waits(nc, max_waits=1):
    """trn2 engine instructions have very few sync-wait slots (TT and DMA
    structs have one). Drop redundant same-engine waits on in-order compute
    engines, then split remaining excess waits onto same-engine NOPs
    inserted immediately before the instruction."""
    eng_map = {
        mybir.EngineType.DVE: nc.vector,
        mybir.EngineType.Activation: nc.scalar,
        mybir.EngineType.Pool: nc.gpsimd,
        mybir.EngineType.PE: nc.tensor,
        mybir.EngineType.SP: nc.sync,
    }
    for fn in nc.m.functions:
        blocks = list(fn.blocks)
        for blk in blocks:
            insts = blk.instructions
            work = []
            for i, ins in enumerate(insts):
                tname = type(ins).__name__
                if tname in _EXEMPT_TYPES:
                    continue
                si = ins.sync_info
                if si is None:
                    continue
                waits = list(si.on_wait)
                eng = ins.engine
                pref = _ENGINE_SEM_PREFIX.get(eng)
                if pref is not None and tname in _SELF_DROP_TYPES:
                    waits = [
                        w for w in waits
                        if not (w.ant_name or "").startswith(pref)
                    ]
                if len(waits) == len(si.on_wait) and len(waits) <= max_waits:
                    continue
                work.append((i, ins, waits))
            for i, ins, waits in reversed(work):
                si = ins.sync_info
                keep, excess = waits[:max_waits], waits[max_waits:]
                ins.sync_info = mybir.SyncInfo(
                    on_wait=keep, on_update=si.on_update
                )
                eng_iface = eng_map[ins.engine]
                for w in reversed(excess):
                    bi = eng_iface.nop(nofuse=True)
                    mi = bi.ins
                    for b2 in fn.blocks:
                        L = b2.instructions
                        for k in range(len(L) - 1, -1, -1):
                            if L[k] is mi or L[k].name == mi.name:
                                del L[k]
                                break
                        else:
                            continue
                        break
                    mi.sync_info = mybir.SyncInfo(on_wait=[w], on_update=[])
                    blk.instructions.insert(i, mi)


def build_kernel(P=128, FPP=8192, F=256, QW=16, RW=32, debug=False,
                 ah_on_pool=False, bh_split=0):
    assert FPP % F == 0
    nchunks = FPP // F
    nc = bass.Bass()

    d_p_ext = nc.declare_dram_parameter("d_pred", [P, FPP], F32, isOutput=False)
    d_g_ext = nc.declare_dram_parameter("d_gt", [P, FPP], F32, isOutput=False)
    m_ext = nc.declare_dram_parameter("mask", [P, FPP], F32, isOutput=False)
    hist_p_ext = nc.declare_dram_parameter("hist_p", [P, RW], F32, isOutput=True)
    hist_g_ext = nc.declare_dram_parameter("hist_g", [P, RW], F32, isOutput=True)
    msum_ext = nc.declare_dram_parameter("msum", [P, nchunks], F32, isOutput=True)
    if debug:
        dbg_q = nc.declare_dram_parameter("dbg_q", [P, F], BF16, isOutput=True)
        dbg_r = nc.declare_dram_parameter("dbg_r", [P, F], BF16, isOutput=True)
        dbg_t = nc.declare_dram_parameter("dbg_t", [P, F], F32, isOutput=True)

    with tile.TileContext(nc) as tc, ExitStack() as ctx:
        singles = ctx.enter_context(tc.tile_pool(name="singles", bufs=1))
        ins = ctx.enter_context(tc.tile_pool(name="ins", bufs=3))
        mids = ctx.enter_context(tc.tile_pool(name="mids", bufs=2))
        hots = ctx.enter_context(tc.tile_pool(name="hots", bufs=2))
        psums = ctx.enter_context(
            tc.tile_pool(name="psums", bufs=2, space=bass.MemorySpace.PSUM)
        )

        # constant one-hot comparison patterns (DVE-built so later DVE
        # readers need no cross-engine wait)
        iota_a = singles.tile([P, QW, F], BF16)
        for w in range(QW):
            nc.vector.memset(iota_a[:, w, :], float(w))
        iota_b = singles.tile([P, RW, F], BF16)
        for w in range(RW):
            nc.vector.memset(iota_b[:, w, :], float(w))

        acc_p = singles.tile([P, RW], F32)
        acc_g = singles.tile([P, RW], F32)
        nc.vector.memset(acc_p, 0.0)
        nc.vector.memset(acc_g, 0.0)
        msum = singles.tile([P, nchunks], F32)

        for c in range(nchunks):
            sl = slice(c * F, (c + 1) * F)
            d_p = ins.tile([P, F], F32, tag="d_p")
            d_g = ins.tile([P, F], F32, tag="d_g")
            m = ins.tile([P, F], F32, tag="m")
            nc.sync.dma_start(out=d_p, in_=d_p_ext[:, sl])
            nc.sync.dma_start(out=d_g, in_=d_g_ext[:, sl])
            nc.sync.dma_start(out=m, in_=m_ext[:, sl])

            # u2 = (4000*(1-m) - guard)/C1
            u0 = mids.tile([P, F], F32, tag="u0")
            nc.vector.tensor_scalar(
                out=u0, in0=m, scalar1=U2_S1, scalar2=None,
                op0=mybir.AluOpType.mult,
            )
            u = mids.tile([P, F], F32, tag="u")
            nc.vector.tensor_scalar(
                out=u, in0=u0, scalar1=U2_S2, scalar2=None,
                op0=mybir.AluOpType.add,
            )
            nc.vector.tensor_reduce(
                out=msum[:, c : c + 1], in_=m, axis=mybir.AxisListType.X,
                op=mybir.AluOpType.add,
            )

            for which, d_t, accum in (("p", d_p, acc_p), ("g", d_g, acc_g)):
                x2 = mids.tile([P, F], F32, tag="x2")
                nc.vector.tensor_tensor(
                    out=x2, in0=d_t, in1=u, op=mybir.AluOpType.add
                )
                # ---- ACT chain: only the first op waits on DVE ----
                t = mids.tile([P, F], F32, tag="t")
                nc.scalar.activation(
                    out=t, in_=x2, func=mybir.ActivationFunctionType.Copy,
                    bias=12582912.0, scale=C1,
                )
                f1 = mids.tile([P, F], F32, tag="f1")
                nc.scalar.activation(
                    out=f1, in_=t, func=mybir.ActivationFunctionType.Copy,
                    bias=-393216.0, scale=0.03125,
                )
                f2 = mids.tile([P, F], F32, tag="f2")
                nc.scalar.activation(
                    out=f2, in_=f1, func=mybir.ActivationFunctionType.Copy,
                    bias=-0.484375, scale=1.0,
                )
                qm = mids.tile([P, F], F32, tag="qm")
                nc.scalar.activation(
                    out=qm, in_=f2, func=mybir.ActivationFunctionType.Copy,
                    bias=12582912.0, scale=1.0,
                )
                q_bf = mids.tile([P, F], BF16, tag="q_bf")
                nc.scalar.activation(
                    out=q_bf, in_=qm, func=mybir.ActivationFunctionType.Copy,
                    bias=-12582912.0, scale=1.0,
                )
                v = mids.tile([P, F], F32, tag="v")
                nc.scalar.activation(
                    out=v, in_=qm, func=mybir.ActivationFunctionType.Copy,
                    bias=-390070272.0, scale=32.0,
                )
                # ---- back to DVE ----
                r_bf = mids.tile([P, F], BF16, tag="r_bf")
                nc.vector.tensor_tensor(
                    out=r_bf, in0=t, in1=v, op=mybir.AluOpType.subtract
                )
                ah = hots.tile([P, QW, F], BF16, tag="ah")
                ah_eng = nc.gpsimd if ah_on_pool else nc.vector
                ah_eng.tensor_tensor(
                    out=ah, in0=q_bf[:, None, :].broadcast_to([P, QW, F]),
                    in1=iota_a, op=mybir.AluOpType.is_equal,
                )
                bh = hots.tile([P, RW, F], BF16, tag="bh")
                if bh_split > 0:
                    k = bh_split
                    nc.gpsimd.tensor_tensor(
                        out=bh[:, :k, :],
                        in0=r_bf[:, None, :].broadcast_to([P, k, F]),
                        in1=iota_b[:, :k, :], op=mybir.AluOpType.is_equal,
                    )
                    nc.vector.tensor_tensor(
                        out=bh[:, k:, :],
                        in0=r_bf[:, None, :].broadcast_to([P, RW - k, F]),
                        in1=iota_b[:, k:, :], op=mybir.AluOpType.is_equal,
                    )
                else:
                    nc.vector.tensor_tensor(
                        out=bh, in0=r_bf[:, None, :].broadcast_to([P, RW, F]),
                        in1=iota_b, op=mybir.AluOpType.is_equal,
                    )

                if debug and c == 0 and which == "p":
                    nc.sync.dma_start(out=dbg_q[:], in_=q_bf)
                    nc.sync.dma_start(out=dbg_r[:], in_=r_bf)
                    nc.sync.dma_start(out=dbg_t[:], in_=t)

                # 3-way PE column-group concurrency: column f accumulates
                # into PSUM partition block 32*(f%3); host sums the 3 blocks.
                # (AP base_partition 96 is not supported, else 4-way.)
                ps = psums.tile([P, RW], F32, tag="ps")
                for f in range(F):
                    j = f % 3
                    nc.tensor.matmul(
                        ps[32 * j : 32 * j + QW, :], ah[:, :, f], bh[:, :, f],
                        start=(f < 3), stop=(f >= F - 3),
                    )
                for j in range(3):
                    sl32 = slice(32 * j, 32 * j + QW)
                    nc.vector.tensor_tensor(
                        out=accum[sl32, :], in0=accum[sl32, :],
                        in1=ps[sl32, :], op=mybir.AluOpType.add,
                    )

        nc.sync.dma_start(out=hist_p_ext[:], in_=acc_p)
        nc.sync.dma_start(out=hist_g_ext[:], in_=acc_g)
        nc.sync.dma_start(out=msum_ext[:], in_=msum)

    legalize_sync_waits(nc)
    return nc



NCORES = 8
P = 128
FPP = 8192  # voxels per partition per core (half a 128^3 volume / 128)
QW, RW = 16, 32

_CACHE = {}


def _get_nc():
    if "nc" not in _CACHE:
        _CACHE["nc"] = build_kernel(P=P, FPP=FPP, F=256, QW=QW, RW=RW)
    return _CACHE["nc"]


def run_device(d_pred, d_gt, mask, trace=False, tmpdir=None):
    """Run the SPMD kernel; returns (results_list, exec_time_ns)."""
    B = d_pred.shape[0]
    V = int(np.prod(d_pred.shape[1:]))
    dp = np.ascontiguousarray(d_pred, dtype=np.float32).reshape(B, V)
    dg = np.ascontiguousarray(d_gt, dtype=np.float32).reshape(B, V)
    mm = np.ascontiguousarray(mask, dtype=np.float32).reshape(B, V)
    half = V // 2
    in_maps = []
    for core in range(NCORES):
        b, h = divmod(core, 2)
        sl = slice(h * half, (h + 1) * half)
        in_maps.append(
            {
                "d_pred": dp[b, sl].reshape(P, FPP),
                "d_gt": dg[b, sl].reshape(P, FPP),
                "mask": mm[b, sl].reshape(P, FPP),
            }
        )
    res = run_bass_kernel_spmd(
        _get_nc(), in_maps, list(range(NCORES)), trace=trace, tmpdir=tmpdir
    )
    return res.results, res.exec_time_ns


def kernel(d_pred, d_gt, mask):
    results, _ = run_device(d_pred, d_gt, mask)
    B = d_pred.shape[0]
    loss = 0.0
    for b in range(B):
        e = np.zeros((QW, RW), np.float64)
        msum = 0.0
        for h in range(2):
            r = results[2 * b + h]
            hp = r["hist_p"].astype(np.float64)
            hg = r["hist_g"].astype(np.float64)
            for j in range(3):
                e += hp[32 * j : 32 * j + QW, :] - hg[32 * j : 32 * j + QW, :]
            msum += float(r["msum"].sum(dtype=np.float64))
        ed = e.reshape(QW * RW)[:500]
        T = np.cumsum(ed[::-1])[::-1]
        denom = msum + 1e-6
        loss += float(np.sum((T / denom) ** 2))
    loss /= B * 500
    return np.float32(loss)



# revision 10
# speedup vs baseline: 2.7193x; 2.7193x over previous
"""DVH global loss (histogram binning) Trainium2 kernel, v2.

Strategy: 8 cores, data-parallel over (batch, voxel-half): core = 2*b + h.
Each core bins 2x 1M voxels (pred, gt) into a 16x32 (q, r) joint histogram,
q = j>>5, r = j&31, j = searchsorted(linspace(0,75,500), d*m, 'right') - 1
computed with exact fp32 magic-number rounding chains.

Engine split (vs v1 which was DVE-bound at 1x broadcast tensor_tensor):
  - index chain: dual-scalar tensor_scalar ops on DVE (fp32 2x_2P mode) +
    one ACT op (v = 32*qm + bias, fma-exact).
  - one-hots: per-row tensor_scalar is_equal (bf16, step-1, even dim ->
    4x_2P mode, 4 elem/cycle/lane), q rows 0..15 and r rows 0..(31-K_TAU)
    on DVE; the last K_TAU r-rows are +-1 thermometer rows built on the
    otherwise-idle ACT engine via Sign(r - (b-0.5)) (decoded on host by
    tail-differencing; exact integer algebra).
  - PE: 8 voxel-columns per matmul via block-strided APs: stationary =
    ah[:, 16a, 8v] (128 cols), moving = bh[:, 32b, 8v] (256 cols), all
    matmuls of one tensor accumulate into a single persistent PSUM
    region [128, 256]; host extracts the 8 diagonal (v == v') blocks.
    2048 LDWEIGHTS+MATMUL pairs total vs 32768 in v1.

Masked-out voxels are shifted past bin 4000 so their q misses the 16-wide
one-hot (their r rows are nonzero but always multiply ah == 0).
mask-sum is recovered on host as the total count in the pred histogram.

A post-Tile pass legalizes semaphore waits (trn2 wait-slot limits).
"""

import sys
from contextlib import ExitStack

if "/opt/trn_rl_repo" not in sys.path:
    sys.path.insert(0, "/opt/trn_rl_repo")

import numpy as np

import concourse.bass as bass
import concourse.tile as tile
from concourse import mybir
from concourse.bass_utils import run_bass_kernel_spmd

F32 = mybir.dt.float32
BF16 = mybir.dt.bfloat16

C1 = 499.0 / 75.0
GUARD = 0.4998
U2_S1 = -4000.0 / C1  # * m
U2_S2 = (4000.0 - GUARD) / C1  # + const
M1 = 12582912.0  # 1.5 * 2^23 fp32 round-to-int magic
M2 = 12582912.0


# trn2 engine instructions have very few sync-wait slots (TT has one). Tile
# emits redundant same-engine waits and multi-waits that walrus rejects.
# Legalize: drop own-engine-sem waits on in-order compute engines, then move
# excess waits onto earlier same-engine instructions with free slots.
_ENGINE_SEM_PREFIX = {
    mybir.EngineType.DVE: "DVE_",
    mybir.EngineType.Activation: "Activation_",
    mybir.EngineType.Pool: "Pool_",
}

_EXEMPT_TYPES = (
    "InstCall",
    "InstUnconditionalBranch",
    "InstRegisterMove",
    "InstISA",
    "InstNoOp",
)

_SELF_DROP_TYPES = (
    "InstTensorTensor",
    "InstTensorScalarPtr",
    "InstTensorScalar",
    "InstTensorReduce",
    "InstActivation",
    "InstMemset",
    "InstTensorCopy",
)


def legalize_sync_waits(nc, max_waits=1):
    """trn2 engine instructions have very few sync-wait slots (TT and DMA
    structs have one). Drop redundant same-engine waits on in-order compute
    engines, then split remaining excess waits onto same-engine NOPs
    inserted immediately before the instruction."""
    eng_map = {
        mybir.EngineType.DVE: nc.vector,
        mybir.EngineType.Activation: nc.scalar,
        mybir.EngineType.Pool: nc.gpsimd,
        mybir.EngineType.PE: nc.tensor,
        mybir.EngineType.SP: nc.sync,
    }
    for fn in nc.m.functions:
        blocks = list(fn.blocks)
        for blk in blocks:
            insts = blk.instructions
            work = []
            for i, ins in enumerate(insts):
                tname = type(ins).__name__
                if tname in _EXEMPT_TYPES:
                    continue
                si = ins.sync_info
                if si is None:
                    continue
                waits = list(si.on_wait)
                eng = ins.engine
                pref = _ENGINE_SEM_PREFIX.get(eng)
                if pref is not None and tname in _SELF_DROP_TYPES:
                    waits = [
                        w for w in waits
                        if not (w.ant_name or "").startswith(pref)
                    ]
                if len(waits) == len(si.on_wait) and len(waits) <= max_waits:
                    continue
                work.append((i, ins, waits))
            for i, ins, waits in reversed(work):
                si = ins.sync_info
                keep, excess = waits[:max_waits], waits[max_waits:]
                ins.sync_info = mybir.SyncInfo(
                    on_wait=keep, on_update=si.on_update
                )
                eng_iface = eng_map[ins.engine]
                for w in reversed(excess):
                    bi = eng_iface.nop(nofuse=True)
                    mi = bi.ins
                    for b2 in fn.blocks:
                        L = b2.instructions
                        for k in range(len(L) - 1, -1, -1):
                            if L[k] is mi or L[k].name == mi.name:
                                del L[k]
                                break
                        else:
                            continue
                        break
                    mi.sync_info = mybir.SyncInfo(on_wait=[w], on_update=[])
                    blk.instructions.insert(i, mi)


def _chunk_list(FPP, V):
    """Split [0, FPP) into chunks of V (last chunk may be smaller,
    multiple of 8)."""
    out = []
    off = 0
    while off < FPP:
        v = min(V, FPP - off)
        assert v % 8 == 0
        out.append((off, v))
        off += v
    return out


def build_kernel(P=128, FPP=8192, V=768, QW=16, RW=32, K_TAU=14, B=8):
    AluOp = mybir.AluOpType
    Act = mybir.ActivationFunctionType
    chunks = _chunk_list(FPP, V)
    n_mm = sum(v // B for _, v in chunks)  # matmuls per tensor

    nc = bass.Bass()
    d_p_ext = nc.declare_dram_parameter("d_pred", [P, FPP], F32, isOutput=False)
    d_g_ext = nc.declare_dram_parameter("d_gt", [P, FPP], F32, isOutput=False)
    m_ext = nc.declare_dram_parameter("mask", [P, FPP], F32, isOutput=False)
    hp_ext = nc.declare_dram_parameter("hist_p", [P, QW * B * 2], F32, isOutput=True)
    hg_ext = nc.declare_dram_parameter("hist_g", [P, QW * B * 2], F32, isOutput=True)

    NOUT = RW * B  # 256 psum columns

    with tile.TileContext(nc) as tc, ExitStack() as ctx:
        singles = ctx.enter_context(tc.tile_pool(name="singles", bufs=1))
        ins = ctx.enter_context(tc.tile_pool(name="ins", bufs=2))
        # chain tiles are produced+consumed in program order on DVE/ACT --
        # single-buffered; q_bf/r_bf feed the (long) one-hot stage and ACT
        # Sign rows, so they get 2 buffers for cross-pass overlap.
        mids = ctx.enter_context(tc.tile_pool(name="mids", bufs=1))
        qrs = ctx.enter_context(tc.tile_pool(name="qrs", bufs=2))
        hots = ctx.enter_context(tc.tile_pool(name="hots", bufs=2))
        psums = ctx.enter_context(
            tc.tile_pool(name="psums", bufs=1, space=bass.MemorySpace.PSUM)
        )

        psum_acc = {
            "p": psums.tile([P, NOUT], F32, name="psum_p", tag="psum_p"),
            "g": psums.tile([P, NOUT], F32, name="psum_g", tag="psum_g"),
        }
        flush_sb = {
            "p": singles.tile([P, NOUT], F32, name="flush_p", tag="flush_p"),
            "g": singles.tile([P, NOUT], F32, name="flush_g", tag="flush_g"),
        }
        mm_count = {"p": 0, "g": 0}

        # per-partition bias columns for the ACT Sign thermometer rows
        if K_TAU > 0:
            tau_bias = singles.tile([P, K_TAU], F32, name="tau_bias", tag="tau_bias")
            for j, b in enumerate(range(RW - K_TAU, RW)):
                nc.vector.memset(tau_bias[:, j : j + 1], 0.5 - float(b))

        for ci, (off, v) in enumerate(chunks):
            sl = slice(off, off + v)
            d_p = ins.tile([P, V], F32, tag="d_p")
            d_g = ins.tile([P, V], F32, tag="d_g")
            m = ins.tile([P, V], F32, tag="m")
            nc.sync.dma_start(out=d_p[:, :v], in_=d_p_ext[:, sl])
            nc.sync.dma_start(out=d_g[:, :v], in_=d_g_ext[:, sl])
            nc.sync.dma_start(out=m[:, :v], in_=m_ext[:, sl])

            # u = m*U2_S1 + U2_S2 (masked-out voxels get +4000-ish dose shift)
            u = mids.tile([P, V], F32, tag="u")
            nc.vector.tensor_scalar(
                out=u[:, :v], in0=m[:, :v],
                scalar1=U2_S1, scalar2=U2_S2,
                op0=AluOp.mult, op1=AluOp.add,
            )

            for which, d_t in (("p", d_p), ("g", d_g)):
                acc = psum_acc[which]
                # x2 = d + u  (fp32, 1x)
                x2 = mids.tile([P, V], F32, tag="x2")
                nc.vector.tensor_tensor(
                    out=x2[:, :v], in0=d_t[:, :v], in1=u[:, :v], op=AluOp.add
                )
                # t = RN(RN(x2*C1) + M1) = j + M1  (magic round; 2x_2P)
                t = mids.tile([P, V], F32, tag="t")
                nc.vector.tensor_scalar(
                    out=t[:, :v], in0=x2[:, :v],
                    scalar1=C1, scalar2=M1,
                    op0=AluOp.mult, op1=AluOp.add,
                )
                # f1 = t/32 - 393216 = j/32 exactly
                f1 = mids.tile([P, V], F32, tag="f1")
                nc.vector.tensor_scalar(
                    out=f1[:, :v], in0=t[:, :v],
                    scalar1=0.03125, scalar2=-393216.0,
                    op0=AluOp.mult, op1=AluOp.add,
                )
                # qm = RN((j/32 - 0.484375) + M2) = q + M2
                qm = mids.tile([P, V], F32, tag="qm")
                nc.vector.tensor_scalar(
                    out=qm[:, :v], in0=f1[:, :v],
                    scalar1=0.484375, scalar2=M2,
                    op0=AluOp.subtract, op1=AluOp.add,
                )
                # q_bf = qm - M2 (bf16; 0..15 valid, 125..140 masked)
                q_bf = qrs.tile([P, V], BF16, tag="q_bf")
                nc.vector.tensor_scalar(
                    out=q_bf[:, :v], in0=qm[:, :v],
                    scalar1=M2, scalar2=None, op0=AluOp.subtract,
                )
                # v32 = 32*qm - 390070272 = 32q + M1 (ACT fma, exact)
                v32 = mids.tile([P, V], F32, tag="v32")
                nc.scalar.activation(
                    out=v32[:, :v], in_=qm[:, :v], func=Act.Copy,
                    bias=-390070272.0, scale=32.0,
                )
                # r_bf = t - v32 = j - 32q (bf16)
                r_bf = qrs.tile([P, V], BF16, tag="r_bf")
                nc.vector.tensor_tensor(
                    out=r_bf[:, :v], in0=t[:, :v], in1=v32[:, :v],
                    op=AluOp.subtract,
                )

                # one-hots (DVE tensor_scalar is_equal, bf16 4x mode),
                # written in PE-ready transposed layout: col = a*B + v so
                # each matmul operand is one contiguous free dim.
                vg = v // B
                q_g = q_bf[:, :v].rearrange("p (g w) -> p g w", w=B)
                r_g = r_bf[:, :v].rearrange("p (g w) -> p g w", w=B)
                ah = hots.tile([P, V // B, QW * B], BF16, tag="ah")
                for a in range(QW):
                    nc.vector.tensor_scalar(
                        out=ah[:, :vg, a * B : (a + 1) * B], in0=q_g,
                        scalar1=float(a), scalar2=None, op0=AluOp.is_equal,
                    )
                bh = hots.tile([P, V // B, RW * B], BF16, tag="bh")
                for b in range(RW - K_TAU):
                    nc.vector.tensor_scalar(
                        out=bh[:, :vg, b * B : (b + 1) * B], in0=r_g,
                        scalar1=float(b), scalar2=None, op0=AluOp.is_equal,
                    )
                # last K_TAU rows: +-1 thermometer via ACT Sign(r - (b-0.5))
                for b in range(RW - K_TAU, RW):
                    j = b - (RW - K_TAU)
                    nc.scalar.activation(
                        out=bh[:, :vg, b * B : (b + 1) * B], in_=r_g,
                        func=Act.Sign,
                        bias=tau_bias[:, j : j + 1], scale=1.0,
                    )

                # PE: 8 voxel-columns per matmul into persistent PSUM
                for g0 in range(vg):
                    i = mm_count[which]
                    nc.tensor.matmul(
                        acc[:, :],
                        ah[:, g0, :],  # [P, 128] stationary, contiguous
                        bh[:, g0, :],  # [P, 256] moving, contiguous
                        start=(i == 0), stop=(i == n_mm - 1),
                    )
                    mm_count[which] += 1

        for which in ("p", "g"):
            nc.vector.tensor_scalar(
                out=flush_sb[which], in0=psum_acc[which],
                scalar1=0.0, scalar2=None, op0=AluOp.add,
            )
        nc.sync.dma_start(out=hp_ext[:], in_=flush_sb["p"])
        nc.sync.dma_start(out=hg_ext[:], in_=flush_sb["g"])

    legalize_sync_waits(nc)
    return nc


NCORES = 8
P = 128
FPP = 8192  # voxels per partition per core (half a 128^3 volume / 128)
QW, RW = 16, 32
K_TAU = 14
V = 768
B = 8

_CACHE = {}


def _get_nc():
    if "nc" not in _CACHE:
        _CACHE["nc"] = build_kernel(
            P=P, FPP=FPP, V=V, QW=QW, RW=RW, K_TAU=K_TAU, B=B
        )
    return _CACHE["nc"]


def decode_hist(M, qw=QW, rw=RW, b=B, k_tau=K_TAU):
    """[128, 256] psum -> [16, 32] integer histogram (float64).

    M[a*8+v, bb*8+v'] : diagonal v==v' blocks hold sum_n ah[a]*row_b[n].
    Rows < rw-k_tau are one-hots; rows >= are +-1 thermometers
    (2*Tail - Cnt)."""
    M4 = M.astype(np.float64).reshape(qw, b, rw, b)
    Hj = np.einsum("avbv->ab", M4)  # [16, 32]
    n_oh = rw - k_tau
    H = np.zeros((qw, rw), np.float64)
    H[:, :n_oh] = Hj[:, :n_oh]
    if k_tau > 0:
        val = Hj[:, n_oh:]  # [16, k_tau]
        cnt = 2.0 * H[:, :n_oh].sum(axis=1) + val[:, 0]  # [16]
        tail = 0.5 * (val + cnt[:, None])  # [16, k_tau]
        H[:, n_oh:-1] = tail[:, :-1] - tail[:, 1:]
        H[:, -1] = tail[:, -1]
    return H


def run_device(d_pred, d_gt, mask, trace=False, tmpdir=None):
    """Run the SPMD kernel; returns (results_list, exec_time_ns)."""
    Bt = d_pred.shape[0]
    Vx = int(np.prod(d_pred.shape[1:]))
    dp = np.ascontiguousarray(d_pred, dtype=np.float32).reshape(Bt, Vx)
    dg = np.ascontiguousarray(d_gt, dtype=np.float32).reshape(Bt, Vx)
    mm = np.ascontiguousarray(mask, dtype=np.float32).reshape(Bt, Vx)
    half = Vx // 2
    in_maps = []
    for core in range(NCORES):
        bb, h = divmod(core, 2)
        sl = slice(h * half, (h + 1) * half)
        in_maps.append(
            {
                "d_pred": dp[bb, sl].reshape(P, FPP),
                "d_gt": dg[bb, sl].reshape(P, FPP),
                "mask": mm[bb, sl].reshape(P, FPP),
            }
        )
    res = run_bass_kernel_spmd(
        _get_nc(), in_maps, list(range(NCORES)), trace=trace, tmpdir=tmpdir
    )
    return res.results, res.exec_time_ns


def kernel(d_pred, d_gt, mask):
    results, _ = run_device(d_pred, d_gt, mask)
    Bt = d_pred.shape[0]
    loss = 0.0
    for bb in range(Bt):
        e = np.zeros((QW, RW), np.float64)
        msum = 0.0
        for h in range(2):
            r = results[2 * bb + h]
            hp = decode_hist(r["hist_p"])
            hg = decode_hist(r["hist_g"])
            e += hp - hg
            msum += float(hp.sum())
        ed = e.reshape(QW * RW)[:500]
        T = np.cumsum(ed[::-1])[::-1]
        denom = msum + 1e-6
        loss += float(np.sum((T / denom) ** 2))
    loss /= Bt * 500
    return np.float32(loss)
